# revision 7
# baseline (speedup 1.0000x reference)
"""Trainium2 Bass kernel for nn_Encoder_conv (conv stack -> big matmuls -> VQ).

Sharding: data-parallel over batch B=16 across 8 cores (2 images/core),
weights replicated, zero cross-core communication, single SPMD launch.

Precision plan (validated against fp64 CPU sim: 0 argmin flips):
  - conv stack: fp32 matmuls (PE 4-pass)
  - mu matmul: fp16 3-pass split (hi + 2^-11-scaled lo), error ~2^-22
  - c matmul:  fp16 single pass (c output tolerance is loose)
  - VQ scores: fp32 matmul, -|e|^2/2 bias folded via ones-row (K=65)
  - gather:    one-hot fp16 matmul (exact 0/1 weights, fp16-rounded emb)
  - loss:      sum over pos of (|lat|^2 - 2*s_max), assembled on host
"""
import sys
import types

import numpy as np

# --- optional NTFF profiling hook shim (antenv.axon_hooks missing in image) ---
try:
    import antenv.axon_hooks  # noqa: F401
except ImportError:
    try:
        import antenv
        _hooks_mod = types.ModuleType("antenv.axon_hooks")
        _hook = [None]
        _hooks_mod.set_axon_ntff_profile_hook = lambda h: _hook.__setitem__(0, h)
        _hooks_mod.get_axon_ntff_profile_hook = lambda: _hook[0]
        sys.modules["antenv.axon_hooks"] = _hooks_mod
        antenv.axon_hooks = _hooks_mod
        from trn_agent_boot.trn_boot import _ntff_profile_via_ctypes
        _hooks_mod.set_axon_ntff_profile_hook(
            _ntff_profile_via_ctypes('/opt/axon/libaxon_pjrt.so'))
    except Exception:
        pass

import concourse.mybir as mybir
import concourse.tile as tile
from concourse import bacc
from concourse.bass_utils import run_bass_kernel_spmd

F32 = mybir.dt.float32
F16 = mybir.dt.float16
U32 = mybir.dt.uint32
AF = mybir.ActivationFunctionType
ALU = mybir.AluOpType

B, H, W = 16, 128, 256
NB = 2          # images per core
NCORES = 8
GH = 32         # hh rows per conv group (4 groups per image)

_NC = None      # cached compiled kernel
TRACE = [False]


def _build_nc():
    nc = bacc.Bacc(target_bir_lowering=False)

    x_d = nc.dram_tensor("x", [NB, H, W], F32, kind="ExternalInput")
    w_in_d = nc.dram_tensor("w_in", [4, 32], F32, kind="ExternalInput")
    w_hd_d = nc.dram_tensor("w_hd", [4, 32, 64], F32, kind="ExternalInput")
    w_hd2_d = nc.dram_tensor("w_hd2", [3, 64, 64], F32, kind="ExternalInput")
    w_r01_d = nc.dram_tensor("w_r01", [3, 64, 256], F32, kind="ExternalInput")
    w_r02_d = nc.dram_tensor("w_r02", [2, 128, 64], F32, kind="ExternalInput")
    w_r11_d = nc.dram_tensor("w_r11", [3, 64, 256], F32, kind="ExternalInput")
    w_r12_d = nc.dram_tensor("w_r12", [2, 128, 64], F32, kind="ExternalInput")
    b_in_d = nc.dram_tensor("b_in", [32, 1], F32, kind="ExternalInput")
    b_hd_d = nc.dram_tensor("b_hd", [64, 1], F32, kind="ExternalInput")
    b_hd2_d = nc.dram_tensor("b_hd2", [64, 1], F32, kind="ExternalInput")
    muwh_d = nc.dram_tensor("muwh", [32, 128, 4096], F16, kind="ExternalInput")
    muwl_d = nc.dram_tensor("muwl", [32, 128, 4096], F16, kind="ExternalInput")
    cw_d = nc.dram_tensor("cw", [32, 128, 4096], F16, kind="ExternalInput")
    embx_d = nc.dram_tensor("embx", [65, 512], F32, kind="ExternalInput")
    embg_d = nc.dram_tensor("embg", [128, 4, 64], F16, kind="ExternalInput")
    kio_d = nc.dram_tensor("kio", [128, 4], F32, kind="ExternalInput")
    id_d = nc.dram_tensor("ident", [128, 128], F32, kind="ExternalInput")

    q_d = nc.dram_tensor("q_out", [NB, H, 4096], F32, kind="ExternalOutput")
    c_d = nc.dram_tensor("c_out", [NB, H, 4096], F32, kind="ExternalOutput")
    sse_d = nc.dram_tensor("sse_out", [4, 64, 8], F32, kind="ExternalOutput")
    smax_d = nc.dram_tensor("smax_out", [4, 128, 32], F32,
                            kind="ExternalOutput")
    idx_scr = nc.dram_tensor("idx_scr", [4, 4096], F16)

    with tile.TileContext(nc) as tc:
        with (
            tc.tile_pool(name="const", bufs=1) as cp,
            tc.tile_pool(name="conv", bufs=1) as vp,
            tc.tile_pool(name="persist", bufs=1) as pp,
            tc.tile_pool(name="stream", bufs=1) as sp,
            tc.tile_pool(name="work", bufs=1) as qp,
            tc.tile_pool(name="ps", bufs=1, space="PSUM") as ps,
        ):
            # ---- constants ----
            w_in = cp.tile([4, 32], F32, tag="w_in")
            nc.sync.dma_start(w_in[:], w_in_d[:])
            w_hd = cp.tile([32, 4, 64], F32, tag="w_hd")
            nc.sync.dma_start(w_hd[:], w_hd_d.ap().rearrange("d k m -> k d m"))
            w_hd2 = cp.tile([64, 3, 64], F32, tag="w_hd2")
            nc.sync.dma_start(w_hd2[:], w_hd2_d.ap().rearrange("d k m -> k d m"))
            w_r01 = cp.tile([64, 3, 256], F32, tag="w_r01")
            nc.sync.dma_start(w_r01[:], w_r01_d.ap().rearrange("d k m -> k d m"))
            w_r02 = cp.tile([128, 2, 64], F32, tag="w_r02")
            nc.sync.dma_start(w_r02[:], w_r02_d.ap().rearrange("t k m -> k t m"))
            w_r11 = cp.tile([64, 3, 256], F32, tag="w_r11")
            nc.sync.dma_start(w_r11[:], w_r11_d.ap().rearrange("d k m -> k d m"))
            w_r12 = cp.tile([128, 2, 64], F32, tag="w_r12")
            nc.sync.dma_start(w_r12[:], w_r12_d.ap().rearrange("t k m -> k t m"))
            b_in = cp.tile([32, 1], F32, tag="b_in")
            nc.sync.dma_start(b_in[:], b_in_d[:])
            b_hd = cp.tile([64, 1], F32, tag="b_hd")
            nc.sync.dma_start(b_hd[:], b_hd_d[:])
            b_hd2 = cp.tile([64, 1], F32, tag="b_hd2")
            nc.sync.dma_start(b_hd2[:], b_hd2_d[:])
            embx = cp.tile([65, 512], F32, tag="embx")
            nc.sync.dma_start(embx[:], embx_d[:])
            embg = cp.tile([128, 4, 64], F16, tag="embg")
            nc.sync.dma_start(embg[:], embg_d[:])
            kio = cp.tile([128, 4], F32, tag="kio")
            nc.sync.dma_start(kio[:], kio_d[:])
            ident = cp.tile([128, 128], F32, tag="ident")
            nc.sync.dma_start(ident[:], id_d[:])
            ones16 = cp.tile([1, 128], F16, tag="ones16")
            nc.vector.memset(ones16[:], 1.0)

            # ---- persistent accumulators ----
            hfT_h = [pp.tile([128, 4096], F16, tag=f"hfTh{p}",
                             name=f"hfTh{p}") for p in range(2)]
            hfT_l = [pp.tile([128, 4096], F16, tag=f"hfTl{p}",
                             name=f"hfTl{p}") for p in range(2)]
            idxcol, smaxcol, sseA = {}, {}, {}
            for b in range(NB):
                for p in range(2):
                    idxcol[(b, p)] = pp.tile([128, 32], F32,
                                             tag=f"idxcol{b}{p}",
                                             name=f"idxcol{b}{p}")
                    smaxcol[(b, p)] = pp.tile([128, 32], F32,
                                              tag=f"smaxcol{b}{p}",
                                              name=f"smaxcol{b}{p}")
                    sseA[(b, p)] = pp.tile([64, 8], F32, tag=f"sseA{b}{p}",
                                           name=f"sseA{b}{p}")

            # =================== conv stack ===================
            for b in range(NB):
                for g in range(4):
                    hh0 = GH * g
                    # --- in-conv (ci=1, k=4, s=2): im2col on partitions ---
                    inx = vp.tile([4, GH * 260], F32, tag="inx")
                    inx_r = inx[:].rearrange("p (h w) -> p h w", h=GH, w=260)
                    nc.vector.memset(inx_r[:, :, 0:4], 0.0)
                    nc.vector.memset(inx_r[:, :, 256:260], 0.0)
                    for dw in range(4):
                        nc.sync.dma_start(
                            inx_r[dw:dw + 1, :, (4 - dw):(4 - dw) + 256],
                            x_d[b:b + 1, hh0:hh0 + GH, :])
                    h1 = vp.tile([32, GH * 130], F32, tag="h1")
                    h1_r = h1[:].rearrange("p (h w) -> p h w", h=GH, w=130)
                    nc.vector.memset(h1_r[:, :, 0:1], 0.0)
                    nc.vector.memset(h1_r[:, :, 129:130], 0.0)
                    for ch in range(8):   # 4 hh x 128 w' = 512
                        psa = ps.tile([32, 512], F32, tag="psA", bufs=2,
                                      name=f"cv1_{b}{g}{ch}")
                        rhs = inx_r[:, 4 * ch:4 * ch + 4, 3:258:2]
                        nc.tensor.matmul(psa[:], w_in[:], rhs,
                                         start=True, stop=True)
                        out = h1_r[:, 4 * ch:4 * ch + 4, 1:129]
                        nc.scalar.activation(out, psa[:], AF.Relu,
                                             bias=b_in[:, 0:1])
                    # --- hd conv (32->64, k=4, s=2) ---
                    h2 = vp.tile([64, GH * 66], F32, tag="hA")
                    h2_r = h2[:].rearrange("p (h w) -> p h w", h=GH, w=66)
                    nc.vector.memset(h2_r[:, :, 0:1], 0.0)
                    nc.vector.memset(h2_r[:, :, 65:66], 0.0)
                    for ch in range(4):   # 8 hh x 64 w' = 512
                        psa = ps.tile([64, 512], F32, tag="psA", bufs=2,
                                      name=f"cv2_{b}{g}{ch}")
                        for dw in range(4):
                            rhs = h1_r[:, 8 * ch:8 * ch + 8, dw:dw + 127:2]
                            nc.tensor.matmul(psa[:], w_hd[:, dw, :], rhs,
                                             start=(dw == 0), stop=(dw == 3))
                        out = h2_r[:, 8 * ch:8 * ch + 8, 1:65]
                        nc.scalar.activation(out, psa[:], AF.Relu,
                                             bias=b_hd[:, 0:1])
                    # --- hd2 conv (64->64, k=3, s=1) ---
                    h3 = vp.tile([64, GH * 66], F32, tag="hB")
                    h3_r = h3[:].rearrange("p (h w) -> p h w", h=GH, w=66)
                    nc.vector.memset(h3_r[:, :, 0:1], 0.0)
                    nc.vector.memset(h3_r[:, :, 65:66], 0.0)
                    for ch in range(4):
                        psa = ps.tile([64, 512], F32, tag="psA", bufs=2,
                                      name=f"cv3_{b}{g}{ch}")
                        for dw in range(3):
                            rhs = h2_r[:, 8 * ch:8 * ch + 8, dw:dw + 64]
                            nc.tensor.matmul(psa[:], w_hd2[:, dw, :], rhs,
                                             start=(dw == 0), stop=(dw == 2))
                        out = h3_r[:, 8 * ch:8 * ch + 8, 1:65]
                        nc.scalar.activation(out, psa[:], AF.Relu,
                                             bias=b_hd2[:, 0:1])
                    # --- residual blocks ---
                    conv_in_r = h3_r     # what the k=3 conv reads
                    res_in_r = h3_r      # pre-relu running h (added at end)
                    for blk, (w1t, w2t) in enumerate(((w_r01, w_r02),
                                                      (w_r11, w_r12))):
                        last = (blk == 1)
                        y1a = vp.tile([128, GH * 64], F32, tag="y1a",
                                      name=f"y1a_{b}{g}{blk}")
                        y1b = vp.tile([128, GH * 64], F32, tag="y1b",
                                      name=f"y1b_{b}{g}{blk}")
                        for ch in range(4):
                            psa = ps.tile([128, 512], F32, tag="psB", bufs=2,
                                          name=f"rw1a_{b}{g}{blk}{ch}")
                            psb_ = ps.tile([128, 512], F32, tag="psC", bufs=2,
                                           name=f"rw1b_{b}{g}{blk}{ch}")
                            for dw in range(3):
                                rhs = conv_in_r[:, 8 * ch:8 * ch + 8,
                                                dw:dw + 64]
                                nc.tensor.matmul(psa[:], w1t[:, dw, 0:128],
                                                 rhs, start=(dw == 0),
                                                 stop=(dw == 2))
                                nc.tensor.matmul(psb_[:], w1t[:, dw, 128:256],
                                                 rhs, start=(dw == 0),
                                                 stop=(dw == 2))
                            nc.scalar.activation(
                                y1a[:, 512 * ch:512 * ch + 512], psa[:],
                                AF.Relu)
                            nc.scalar.activation(
                                y1b[:, 512 * ch:512 * ch + 512], psb_[:],
                                AF.Relu)
                        if not last:
                            hout = vp.tile([64, GH * 66], F32, tag="hA",
                                           name=f"h4_{b}{g}")
                            hout_r = hout[:].rearrange("p (h w) -> p h w",
                                                       h=GH, w=66)
                            nc.vector.memset(hout_r[:, :, 0:1], 0.0)
                            nc.vector.memset(hout_r[:, :, 65:66], 0.0)
                            hrelu = vp.tile([64, GH * 66], F32, tag="hC",
                                            name=f"h4r_{b}{g}")
                            hrelu_r = hrelu[:].rearrange("p (h w) -> p h w",
                                                         h=GH, w=66)
                            nc.vector.memset(hrelu_r[:, :, 0:1], 0.0)
                            nc.vector.memset(hrelu_r[:, :, 65:66], 0.0)
                        else:
                            h5r = vp.tile([64, GH * 64], F32, tag="h5r",
                                          name=f"h5r_{b}{g}")
                        for ch in range(4):
                            psa = ps.tile([64, 512], F32, tag="psA", bufs=2,
                                          name=f"rw2_{b}{g}{blk}{ch}")
                            nc.tensor.matmul(psa[:], w2t[:, 0, :],
                                             y1a[:, 512 * ch:512 * ch + 512],
                                             start=True, stop=False)
                            nc.tensor.matmul(psa[:], w2t[:, 1, :],
                                             y1b[:, 512 * ch:512 * ch + 512],
                                             start=False, stop=True)
                            psv = psa[:].rearrange("p (a w) -> p a w",
                                                   a=8, w=64)
                            hin_i = res_in_r[:, 8 * ch:8 * ch + 8, 1:65]
                            if not last:
                                out = hout_r[:, 8 * ch:8 * ch + 8, 1:65]
                                nc.vector.tensor_tensor(out, psv, hin_i,
                                                        ALU.add)
                                outr = hrelu_r[:, 8 * ch:8 * ch + 8, 1:65]
                                nc.scalar.activation(outr, out, AF.Relu)
                            else:
                                out = h5r[:, 512 * ch:512 * ch + 512]
                                outv = out.rearrange("p (a w) -> p a w",
                                                     a=8, w=64)
                                nc.vector.tensor_tensor(outv, psv, hin_i,
                                                        ALU.add)
                                nc.scalar.activation(out, out, AF.Relu)
                        if not last:
                            conv_in_r = hrelu_r   # r1 conv reads relu(h4)
                            res_in_r = hout_r     # r1 residual adds h4
                    # --- transposes for this group -> hfT (p = g//2) ---
                    p = g // 2
                    j0 = 16 * (g % 2)
                    for t in range(16):
                        psT = ps.tile([128, 64], F32, tag="psD", bufs=2,
                                      name=f"psT_{b}{g}{t}")
                        nc.tensor.transpose(psT[:],
                                            h5r[:, 128 * t:128 * t + 128],
                                            ident[0:64, 0:64])
                        j = j0 + t
                        dsth = hfT_h[p][:, 128 * j + 64 * b:
                                        128 * j + 64 * b + 64]
                        dstl = hfT_l[p][:, 128 * j + 64 * b:
                                        128 * j + 64 * b + 64]
                        nc.scalar.activation(dsth, psT[:], AF.Copy)
                        tmp = qp.tile([128, 64], F32, tag="splittmp", bufs=2,
                                      name=f"spl_{b}{g}{t}")
                        nc.vector.tensor_tensor(tmp[:], psT[:], dsth,
                                                ALU.subtract)
                        nc.vector.tensor_scalar(dstl, tmp[:], 2048.0, None,
                                                ALU.mult)

            # ============ mu matmul (fp16 x3) + VQ scores per o-chunk ======
            for oc in range(8):
                mains = [ps.tile([128, 512], F32, tag="psA", bufs=2,
                                 name=f"mumain{p}_{oc}") for p in range(2)]
                crosses = [ps.tile([128, 512], F32, tag="psB", bufs=2,
                                   name=f"mucross{p}_{oc}") for p in range(2)]
                for j in range(32):
                    wh_t = sp.tile([128, 512], F16, tag="wh", bufs=4,
                                   name=f"wh_{oc}_{j}")
                    nc.sync.dma_start(wh_t[:],
                                      muwh_d[j, :, 512 * oc:512 * oc + 512])
                    wl_t = sp.tile([128, 512], F16, tag="wl", bufs=4,
                                   name=f"wl_{oc}_{j}")
                    nc.sync.dma_start(wl_t[:],
                                      muwl_d[j, :, 512 * oc:512 * oc + 512])
                    for p in range(2):
                        lh = hfT_h[p][:, 128 * j:128 * j + 128]
                        ll = hfT_l[p][:, 128 * j:128 * j + 128]
                        nc.tensor.matmul(mains[p][:], lh, wh_t[:],
                                         start=(j == 0), stop=(j == 31))
                        nc.tensor.matmul(crosses[p][:], lh, wl_t[:],
                                         start=(j == 0), stop=False)
                        nc.tensor.matmul(crosses[p][:], ll, wh_t[:],
                                         start=False, stop=(j == 31))
                for p in range(2):
                    tmp = qp.tile([128, 512], F32, tag="crosstmp", bufs=2,
                                  name=f"ctm_{oc}{p}")
                    nc.vector.tensor_scalar(tmp[:], crosses[p][:],
                                            1.0 / 2048.0, None, ALU.mult)
                    for b in range(NB):
                        msub = qp.tile([65, 512], F32, tag="msub", bufs=8,
                                       name=f"msub{b}{p}_{oc}")
                        nc.vector.memset(msub[64:65, :], 1.0)
                        nc.vector.tensor_tensor(
                            msub[0:64, :],
                            mains[p][64 * b:64 * b + 64, :],
                            tmp[64 * b:64 * b + 64, :], ALU.add)
                        # |lat|^2 accumulation (row sums of squares)
                        sqt = qp.tile([64, 512], F32, tag="sqt", bufs=2,
                                      name=f"sqt{b}{p}_{oc}")
                        nc.scalar.activation(
                            sqt[:], msub[0:64, :], AF.Square,
                            accum_out=sseA[(b, p)][:, oc:oc + 1])
                        # scores + argmax for the 4 pos-chunks of this oc
                        for sc in range(4):
                            pss = ps.tile([128, 512], F32, tag="psC", bufs=2,
                                          name=f"pss{b}{p}{oc}{sc}")
                            nc.tensor.matmul(
                                pss[:], msub[0:65, 128 * sc:128 * sc + 128],
                                embx[0:65, :], start=True, stop=True)
                            vmax = qp.tile([128, 8], F32, tag="vmax", bufs=2,
                                           name=f"vmax{b}{p}{oc}{sc}")
                            vidx = qp.tile([128, 8], U32, tag="vidx", bufs=2,
                                           name=f"vidx{b}{p}{oc}{sc}")
                            nc.vector.max_with_indices(vmax[:], vidx[:],
                                                       pss[:])
                            cidx = 4 * oc + sc
                            nc.vector.tensor_copy(
                                idxcol[(b, p)][:, cidx:cidx + 1],
                                vidx[:, 0:1])
                            nc.vector.tensor_copy(
                                smaxcol[(b, p)][:, cidx:cidx + 1],
                                vmax[:, 0:1])

            # =================== c matmul (fp16 x1) ===================
            for oc in range(8):
                pcs = [ps.tile([128, 512], F32, tag="psA", bufs=2,
                               name=f"cpsm{p}_{oc}") for p in range(2)]
                for j in range(32):
                    cw_t = sp.tile([128, 512], F16, tag="cw", bufs=4,
                                   name=f"cw_{oc}_{j}")
                    nc.sync.dma_start(cw_t[:],
                                      cw_d[j, :, 512 * oc:512 * oc + 512])
                    for p in range(2):
                        lh = hfT_h[p][:, 128 * j:128 * j + 128]
                        nc.tensor.matmul(pcs[p][:], lh, cw_t[:],
                                         start=(j == 0), stop=(j == 31))
                for p in range(2):
                    for b in range(NB):
                        ctmp = qp.tile([64, 512], F32, tag="ctmp", bufs=2,
                                       name=f"ctp{b}{p}_{oc}")
                        nc.scalar.activation(ctmp[:],
                                             pcs[p][64 * b:64 * b + 64, :],
                                             AF.Copy)
                        dst = c_d.ap().rearrange(
                            "b (h two) o -> b h two o",
                            two=2)[b, :, p, 512 * oc:512 * oc + 512]
                        nc.gpsimd.dma_start(dst, ctmp[:])

            # =================== gather (quantized output) ===============
            for b in range(NB):
                for p in range(2):
                    bp = 2 * b + p
                    psT2 = ps.tile([32, 128], F32, tag="psD", bufs=2,
                                   name=f"psT2_{bp}")
                    nc.tensor.transpose(psT2[:], idxcol[(b, p)][:], ident[:])
                    idxT16 = qp.tile([32, 128], F16, tag="idxT16", bufs=2,
                                     name=f"idxT16_{bp}")
                    nc.vector.tensor_copy(idxT16[:], psT2[:])
                    nc.gpsimd.dma_start(idx_scr[bp, :], idxT16[:])
                    idxrow16 = qp.tile([1, 4096], F16, tag="idxrow16",
                                       name=f"idxrow16_{bp}")
                    nc.gpsimd.dma_start(idxrow16[:], idx_scr[bp:bp + 1, :])
                    nc.gpsimd.dma_start(smax_d[bp, :, :], smaxcol[(b, p)][:])
                    nc.gpsimd.dma_start(sse_d[bp, :, :], sseA[(b, p)][:])
                    for c2 in range(8):
                        psb = ps.tile([128, 512], F32, tag="psC", bufs=2,
                                      name=f"psbq{bp}{c2}")
                        nc.tensor.matmul(
                            psb[:], ones16[:],
                            idxrow16[0:1, 512 * c2:512 * c2 + 512],
                            start=True, stop=True)
                        qps = ps.tile([64, 512], F32, tag="psD", bufs=2,
                                      name=f"qps{bp}{c2}")
                        for kt in range(4):
                            oh = qp.tile([128, 512], F16, tag="oh", bufs=2,
                                         name=f"oh{bp}{c2}{kt}")
                            nc.vector.tensor_scalar(oh[:], psb[:],
                                                    kio[:, kt:kt + 1], None,
                                                    ALU.is_equal)
                            nc.tensor.matmul(qps[:], embg[:, kt, :], oh[:],
                                             start=(kt == 0), stop=(kt == 3))
                        qtmp = qp.tile([64, 512], F32, tag="qtmp", bufs=2,
                                       name=f"qtmp{bp}{c2}")
                        nc.scalar.activation(qtmp[:], qps[:], AF.Copy)
                        dst = q_d.ap().rearrange(
                            "b (h two) o -> b h two o",
                            two=2)[b, :, p, 512 * c2:512 * c2 + 512]
                        nc.gpsimd.dma_start(dst, qtmp[:])

    nc.compile()
    return nc


def _prep_inputs(x, params):
    p = {k: np.asarray(v, np.float32) for k, v in params.items()}
    assert not np.any(p["mu_b"]) and not np.any(p["c_b"]), \
        "nonzero mu_b/c_b not supported"

    muT = np.ascontiguousarray(p["mu_w"].T)            # [4096 i, 4096 o]
    muwh = muT.astype(np.float16)
    muwl = ((muT - muwh.astype(np.float32)) * 2048.0).astype(np.float16)
    cT = np.ascontiguousarray(p["c_w"].T).astype(np.float16)
    emb = p["emb"]                                     # [512, 64]
    embx = np.concatenate([emb.T, -0.5 * np.sum(emb * emb, 1)[None, :]],
                          axis=0).astype(np.float32)   # [65, 512]
    embg = np.ascontiguousarray(
        emb.reshape(4, 128, 64).transpose(1, 0, 2)).astype(np.float16)
    kio = np.ascontiguousarray(
        np.arange(128, dtype=np.float32)[:, None]
        + 128.0 * np.arange(4, dtype=np.float32)[None, :])

    shared = {
        "w_in": np.ascontiguousarray(p["in_w"][:, 0, 0, :].T),       # [4, 32]
        "w_hd": np.ascontiguousarray(p["hd_w"][:, :, 0, :].transpose(2, 1, 0)),
        "w_hd2": np.ascontiguousarray(p["hd2_w"][:, :, 0, :].transpose(2, 1, 0)),
        "w_r01": np.ascontiguousarray(p["r0_w1"][:, :, 0, :].transpose(2, 1, 0)),
        "w_r02": np.ascontiguousarray(
            p["r0_w2"][:, :, 0, 0].T.reshape(2, 128, 64)),
        "w_r11": np.ascontiguousarray(p["r1_w1"][:, :, 0, :].transpose(2, 1, 0)),
        "w_r12": np.ascontiguousarray(
            p["r1_w2"][:, :, 0, 0].T.reshape(2, 128, 64)),
        "b_in": np.ascontiguousarray(p["in_b"][:, None]),
        "b_hd": np.ascontiguousarray(p["hd_b"][:, None]),
        "b_hd2": np.ascontiguousarray(p["hd2_b"][:, None]),
        "muwh": np.ascontiguousarray(muwh.reshape(32, 128, 4096)),
        "muwl": np.ascontiguousarray(muwl.reshape(32, 128, 4096)),
        "cw": np.ascontiguousarray(cT.reshape(32, 128, 4096)),
        "embx": embx,
        "embg": embg,
        "kio": kio,
        "ident": np.eye(128, dtype=np.float32),
    }
    x = np.asarray(x, np.float32)
    in_maps = []
    for k in range(NCORES):
        m = dict(shared)
        m["x"] = np.ascontiguousarray(x[2 * k:2 * k + 2])
        in_maps.append(m)
    return in_maps


def kernel(x, params):
    global _NC
    if _NC is None:
        _NC = _build_nc()
    in_maps = _prep_inputs(x, params)
    res = run_bass_kernel_spmd(_NC, in_maps, list(range(NCORES)),
                               trace=TRACE[0])
    kernel.last_exec_time_ns = res.exec_time_ns
    q = np.concatenate([res.results[k]["q_out"] for k in range(NCORES)],
                       axis=0)
    c = np.concatenate([res.results[k]["c_out"] for k in range(NCORES)],
                       axis=0)
    sse = 0.0
    for k in range(NCORES):
        sse += float(res.results[k]["sse_out"].sum())
        sse -= 2.0 * float(res.results[k]["smax_out"].sum())
    loss = np.float32(2.0 * sse / (B * H * 64 * 64))
    return q, loss, c


# revision 8
# speedup vs baseline: 1.0069x; 1.0069x over previous
"""Trainium2 Bass kernel for nn_Encoder_conv (conv stack -> big matmuls -> VQ).

Sharding: data-parallel over batch B=16 across 8 cores (2 images/core),
weights replicated, zero cross-core communication, single SPMD launch.

Precision plan (validated against fp64 CPU sim: 0 argmin flips):
  - conv stack: fp32 matmuls (PE 4-pass)
  - mu matmul: fp16 3-pass split (hi + 2^-11-scaled lo), error ~2^-22
  - c matmul:  fp16 single pass (c output tolerance is loose)
  - VQ scores: fp32 matmul, -|e|^2/2 bias folded via ones-row (K=65)
  - gather:    one-hot fp16 matmul (exact 0/1 weights, fp16-rounded emb)
  - loss:      sum over pos of (|lat|^2 - 2*s_max), assembled on host
"""
import sys
import types

import numpy as np

# --- optional NTFF profiling hook shim (antenv.axon_hooks missing in image) ---
try:
    import antenv.axon_hooks  # noqa: F401
except ImportError:
    try:
        import antenv
        _hooks_mod = types.ModuleType("antenv.axon_hooks")
        _hook = [None]
        _hooks_mod.set_axon_ntff_profile_hook = lambda h: _hook.__setitem__(0, h)
        _hooks_mod.get_axon_ntff_profile_hook = lambda: _hook[0]
        sys.modules["antenv.axon_hooks"] = _hooks_mod
        antenv.axon_hooks = _hooks_mod
        from trn_agent_boot.trn_boot import _ntff_profile_via_ctypes
        _hooks_mod.set_axon_ntff_profile_hook(
            _ntff_profile_via_ctypes('/opt/axon/libaxon_pjrt.so'))
    except Exception:
        pass

import concourse.mybir as mybir
import concourse.tile as tile
from concourse import bacc
from concourse.bass_utils import run_bass_kernel_spmd

F32 = mybir.dt.float32
F16 = mybir.dt.float16
U32 = mybir.dt.uint32
AF = mybir.ActivationFunctionType
ALU = mybir.AluOpType

B, H, W = 16, 128, 256
NB = 2          # images per core
NCORES = 8
GH = 32         # hh rows per conv group (4 groups per image)

_NC = None      # cached compiled kernel
TRACE = [False]


def _build_nc():
    nc = bacc.Bacc(target_bir_lowering=False)

    x_d = nc.dram_tensor("x", [NB, H, W], F32, kind="ExternalInput")
    w_in_d = nc.dram_tensor("w_in", [4, 32], F32, kind="ExternalInput")
    w_hd_d = nc.dram_tensor("w_hd", [4, 32, 64], F32, kind="ExternalInput")
    w_hd2_d = nc.dram_tensor("w_hd2", [3, 64, 64], F32, kind="ExternalInput")
    w_r01_d = nc.dram_tensor("w_r01", [3, 64, 256], F32, kind="ExternalInput")
    w_r02_d = nc.dram_tensor("w_r02", [2, 128, 64], F32, kind="ExternalInput")
    w_r11_d = nc.dram_tensor("w_r11", [3, 64, 256], F32, kind="ExternalInput")
    w_r12_d = nc.dram_tensor("w_r12", [2, 128, 64], F32, kind="ExternalInput")
    b_in_d = nc.dram_tensor("b_in", [32, 1], F32, kind="ExternalInput")
    b_hd_d = nc.dram_tensor("b_hd", [64, 1], F32, kind="ExternalInput")
    b_hd2_d = nc.dram_tensor("b_hd2", [64, 1], F32, kind="ExternalInput")
    muwh_d = nc.dram_tensor("muwh", [32, 128, 4096], F16, kind="ExternalInput")
    muwl_d = nc.dram_tensor("muwl", [32, 128, 4096], F16, kind="ExternalInput")
    cw_d = nc.dram_tensor("cw", [32, 128, 4096], F16, kind="ExternalInput")
    embx_d = nc.dram_tensor("embx", [65, 512], F32, kind="ExternalInput")
    embg_d = nc.dram_tensor("embg", [128, 4, 64], F16, kind="ExternalInput")
    kio_d = nc.dram_tensor("kio", [128, 4], F32, kind="ExternalInput")
    id_d = nc.dram_tensor("ident", [128, 128], F32, kind="ExternalInput")

    q_d = nc.dram_tensor("q_out", [NB, H, 4096], F32, kind="ExternalOutput")
    c_d = nc.dram_tensor("c_out", [NB, H, 4096], F32, kind="ExternalOutput")
    sse_d = nc.dram_tensor("sse_out", [4, 64, 8], F32, kind="ExternalOutput")
    smax_d = nc.dram_tensor("smax_out", [4, 128, 32], F32,
                            kind="ExternalOutput")
    idx_scr = nc.dram_tensor("idx_scr", [4, 4096], F16)

    with tile.TileContext(nc) as tc:
        with (
            tc.tile_pool(name="const", bufs=1) as cp,
            tc.tile_pool(name="conv", bufs=1) as vp,
            tc.tile_pool(name="persist", bufs=1) as pp,
            tc.tile_pool(name="stream", bufs=1) as sp,
            tc.tile_pool(name="work", bufs=1) as qp,
            tc.tile_pool(name="ps", bufs=1, space="PSUM") as ps,
        ):
            # ---- constants ----
            w_in = cp.tile([4, 32], F32, tag="w_in")
            nc.sync.dma_start(w_in[:], w_in_d[:])
            w_hd = cp.tile([32, 4, 64], F32, tag="w_hd")
            nc.sync.dma_start(w_hd[:], w_hd_d.ap().rearrange("d k m -> k d m"))
            w_hd2 = cp.tile([64, 3, 64], F32, tag="w_hd2")
            nc.sync.dma_start(w_hd2[:], w_hd2_d.ap().rearrange("d k m -> k d m"))
            w_r01 = cp.tile([64, 3, 256], F32, tag="w_r01")
            nc.sync.dma_start(w_r01[:], w_r01_d.ap().rearrange("d k m -> k d m"))
            w_r02 = cp.tile([128, 2, 64], F32, tag="w_r02")
            nc.sync.dma_start(w_r02[:], w_r02_d.ap().rearrange("t k m -> k t m"))
            w_r11 = cp.tile([64, 3, 256], F32, tag="w_r11")
            nc.sync.dma_start(w_r11[:], w_r11_d.ap().rearrange("d k m -> k d m"))
            w_r12 = cp.tile([128, 2, 64], F32, tag="w_r12")
            nc.sync.dma_start(w_r12[:], w_r12_d.ap().rearrange("t k m -> k t m"))
            b_in = cp.tile([32, 1], F32, tag="b_in")
            nc.sync.dma_start(b_in[:], b_in_d[:])
            b_hd = cp.tile([64, 1], F32, tag="b_hd")
            nc.sync.dma_start(b_hd[:], b_hd_d[:])
            b_hd2 = cp.tile([64, 1], F32, tag="b_hd2")
            nc.sync.dma_start(b_hd2[:], b_hd2_d[:])
            embx = cp.tile([65, 512], F32, tag="embx")
            nc.sync.dma_start(embx[:], embx_d[:])
            embg = cp.tile([128, 4, 64], F16, tag="embg")
            nc.sync.dma_start(embg[:], embg_d[:])
            kio = cp.tile([128, 4], F32, tag="kio")
            nc.sync.dma_start(kio[:], kio_d[:])
            ident = cp.tile([128, 128], F32, tag="ident")
            nc.sync.dma_start(ident[:], id_d[:])
            ones16 = cp.tile([1, 128], F16, tag="ones16")
            nc.vector.memset(ones16[:], 1.0)

            # ---- persistent accumulators ----
            hfT_h = [pp.tile([128, 4096], F16, tag=f"hfTh{p}",
                             name=f"hfTh{p}") for p in range(2)]
            hfT_l = [pp.tile([128, 4096], F16, tag=f"hfTl{p}",
                             name=f"hfTl{p}") for p in range(2)]
            idxcol, smaxcol, sseA = {}, {}, {}
            for b in range(NB):
                for p in range(2):
                    idxcol[(b, p)] = pp.tile([128, 32], F32,
                                             tag=f"idxcol{b}{p}",
                                             name=f"idxcol{b}{p}")
                    smaxcol[(b, p)] = pp.tile([128, 32], F32,
                                              tag=f"smaxcol{b}{p}",
                                              name=f"smaxcol{b}{p}")
                    sseA[(b, p)] = pp.tile([64, 8], F32, tag=f"sseA{b}{p}",
                                           name=f"sseA{b}{p}")

            # =================== conv stack ===================
            for b in range(NB):
                for g in range(4):
                    hh0 = GH * g
                    # --- in-conv (ci=1, k=4, s=2): im2col on partitions ---
                    inx = vp.tile([4, GH * 260], F32, tag="inx")
                    inx_r = inx[:].rearrange("p (h w) -> p h w", h=GH, w=260)
                    nc.vector.memset(inx_r[:, :, 0:4], 0.0)
                    nc.vector.memset(inx_r[:, :, 256:260], 0.0)
                    for dw in range(4):
                        nc.sync.dma_start(
                            inx_r[dw:dw + 1, :, (4 - dw):(4 - dw) + 256],
                            x_d[b:b + 1, hh0:hh0 + GH, :])
                    h1 = vp.tile([32, GH * 130], F32, tag="h1")
                    h1_r = h1[:].rearrange("p (h w) -> p h w", h=GH, w=130)
                    nc.vector.memset(h1_r[:, :, 0:1], 0.0)
                    nc.vector.memset(h1_r[:, :, 129:130], 0.0)
                    for ch in range(8):   # 4 hh x 128 w' = 512
                        psa = ps.tile([32, 512], F32, tag="psA", bufs=2,
                                      name=f"cv1_{b}{g}{ch}")
                        rhs = inx_r[:, 4 * ch:4 * ch + 4, 3:258:2]
                        nc.tensor.matmul(psa[:], w_in[:], rhs,
                                         start=True, stop=True)
                        out = h1_r[:, 4 * ch:4 * ch + 4, 1:129]
                        nc.scalar.activation(out, psa[:], AF.Relu,
                                             bias=b_in[:, 0:1])
                    # --- hd conv (32->64, k=4, s=2) ---
                    h2 = vp.tile([64, GH * 66], F32, tag="hA")
                    h2_r = h2[:].rearrange("p (h w) -> p h w", h=GH, w=66)
                    nc.vector.memset(h2_r[:, :, 0:1], 0.0)
                    nc.vector.memset(h2_r[:, :, 65:66], 0.0)
                    for ch in range(4):   # 8 hh x 64 w' = 512
                        psa = ps.tile([64, 512], F32, tag="psA", bufs=2,
                                      name=f"cv2_{b}{g}{ch}")
                        for dw in range(4):
                            rhs = h1_r[:, 8 * ch:8 * ch + 8, dw:dw + 127:2]
                            nc.tensor.matmul(psa[:], w_hd[:, dw, :], rhs,
                                             start=(dw == 0), stop=(dw == 3))
                        out = h2_r[:, 8 * ch:8 * ch + 8, 1:65]
                        nc.scalar.activation(out, psa[:], AF.Relu,
                                             bias=b_hd[:, 0:1])
                    # --- hd2 conv (64->64, k=3, s=1) ---
                    h3 = vp.tile([64, GH * 66], F32, tag="hB")
                    h3_r = h3[:].rearrange("p (h w) -> p h w", h=GH, w=66)
                    nc.vector.memset(h3_r[:, :, 0:1], 0.0)
                    nc.vector.memset(h3_r[:, :, 65:66], 0.0)
                    for ch in range(4):
                        psa = ps.tile([64, 512], F32, tag="psA", bufs=2,
                                      name=f"cv3_{b}{g}{ch}")
                        for dw in range(3):
                            rhs = h2_r[:, 8 * ch:8 * ch + 8, dw:dw + 64]
                            nc.tensor.matmul(psa[:], w_hd2[:, dw, :], rhs,
                                             start=(dw == 0), stop=(dw == 2))
                        out = h3_r[:, 8 * ch:8 * ch + 8, 1:65]
                        nc.scalar.activation(out, psa[:], AF.Relu,
                                             bias=b_hd2[:, 0:1])
                    # --- residual blocks ---
                    conv_in_r = h3_r     # what the k=3 conv reads
                    res_in_r = h3_r      # pre-relu running h (added at end)
                    for blk, (w1t, w2t) in enumerate(((w_r01, w_r02),
                                                      (w_r11, w_r12))):
                        last = (blk == 1)
                        y1a = vp.tile([128, GH * 64], F32, tag="y1a",
                                      name=f"y1a_{b}{g}{blk}")
                        y1b = vp.tile([128, GH * 64], F32, tag="y1b",
                                      name=f"y1b_{b}{g}{blk}")
                        for ch in range(4):
                            psa = ps.tile([128, 512], F32, tag="psB", bufs=2,
                                          name=f"rw1a_{b}{g}{blk}{ch}")
                            psb_ = ps.tile([128, 512], F32, tag="psC", bufs=2,
                                           name=f"rw1b_{b}{g}{blk}{ch}")
                            for dw in range(3):
                                rhs = conv_in_r[:, 8 * ch:8 * ch + 8,
                                                dw:dw + 64]
                                nc.tensor.matmul(psa[:], w1t[:, dw, 0:128],
                                                 rhs, start=(dw == 0),
                                                 stop=(dw == 2))
                                nc.tensor.matmul(psb_[:], w1t[:, dw, 128:256],
                                                 rhs, start=(dw == 0),
                                                 stop=(dw == 2))
                            nc.scalar.activation(
                                y1a[:, 512 * ch:512 * ch + 512], psa[:],
                                AF.Relu)
                            nc.scalar.activation(
                                y1b[:, 512 * ch:512 * ch + 512], psb_[:],
                                AF.Relu)
                        if not last:
                            hout = vp.tile([64, GH * 66], F32, tag="hA",
                                           name=f"h4_{b}{g}")
                            hout_r = hout[:].rearrange("p (h w) -> p h w",
                                                       h=GH, w=66)
                            nc.vector.memset(hout_r[:, :, 0:1], 0.0)
                            nc.vector.memset(hout_r[:, :, 65:66], 0.0)
                            hrelu = vp.tile([64, GH * 66], F32, tag="hC",
                                            name=f"h4r_{b}{g}")
                            hrelu_r = hrelu[:].rearrange("p (h w) -> p h w",
                                                         h=GH, w=66)
                            nc.vector.memset(hrelu_r[:, :, 0:1], 0.0)
                            nc.vector.memset(hrelu_r[:, :, 65:66], 0.0)
                        else:
                            h5r = vp.tile([64, GH * 64], F32, tag="h5r",
                                          name=f"h5r_{b}{g}")
                        for ch in range(4):
                            psa = ps.tile([64, 512], F32, tag="psA", bufs=2,
                                          name=f"rw2_{b}{g}{blk}{ch}")
                            nc.tensor.matmul(psa[:], w2t[:, 0, :],
                                             y1a[:, 512 * ch:512 * ch + 512],
                                             start=True, stop=False)
                            nc.tensor.matmul(psa[:], w2t[:, 1, :],
                                             y1b[:, 512 * ch:512 * ch + 512],
                                             start=False, stop=True)
                            psv = psa[:].rearrange("p (a w) -> p a w",
                                                   a=8, w=64)
                            hin_i = res_in_r[:, 8 * ch:8 * ch + 8, 1:65]
                            if not last:
                                out = hout_r[:, 8 * ch:8 * ch + 8, 1:65]
                                nc.vector.tensor_tensor(out, psv, hin_i,
                                                        ALU.add)
                                outr = hrelu_r[:, 8 * ch:8 * ch + 8, 1:65]
                                nc.scalar.activation(outr, out, AF.Relu)
                            else:
                                out = h5r[:, 512 * ch:512 * ch + 512]
                                outv = out.rearrange("p (a w) -> p a w",
                                                     a=8, w=64)
                                nc.vector.tensor_tensor(outv, psv, hin_i,
                                                        ALU.add)
                                nc.scalar.activation(out, out, AF.Relu)
                        if not last:
                            conv_in_r = hrelu_r   # r1 conv reads relu(h4)
                            res_in_r = hout_r     # r1 residual adds h4
                    # --- transposes for this group -> hfT (p = g//2) ---
                    p = g // 2
                    j0 = 16 * (g % 2)
                    for t in range(16):
                        psT = ps.tile([128, 64], F32, tag="psD", bufs=2,
                                      name=f"psT_{b}{g}{t}")
                        nc.tensor.transpose(psT[:],
                                            h5r[:, 128 * t:128 * t + 128],
                                            ident[0:64, 0:64])
                        j = j0 + t
                        dsth = hfT_h[p][:, 128 * j + 64 * b:
                                        128 * j + 64 * b + 64]
                        dstl = hfT_l[p][:, 128 * j + 64 * b:
                                        128 * j + 64 * b + 64]
                        nc.scalar.activation(dsth, psT[:], AF.Copy)
                        tmp = qp.tile([128, 64], F32, tag="splittmp", bufs=2,
                                      name=f"spl_{b}{g}{t}")
                        nc.vector.tensor_tensor(tmp[:], psT[:], dsth,
                                                ALU.subtract)
                        nc.vector.tensor_scalar(dstl, tmp[:], 2048.0, None,
                                                ALU.mult)

            # ============ mu matmul (fp16 x3) + VQ scores per o-chunk ======
            for oc in range(8):
                mains = [ps.tile([128, 512], F32, tag="psA", bufs=2,
                                 name=f"mumain{p}_{oc}") for p in range(2)]
                crosses = [ps.tile([128, 512], F32, tag="psB", bufs=2,
                                   name=f"mucross{p}_{oc}") for p in range(2)]
                for j in range(32):
                    wh_t = sp.tile([128, 512], F16, tag="wh", bufs=4,
                                   name=f"wh_{oc}_{j}")
                    nc.sync.dma_start(wh_t[:],
                                      muwh_d[j, :, 512 * oc:512 * oc + 512])
                    wl_t = sp.tile([128, 512], F16, tag="wl", bufs=4,
                                   name=f"wl_{oc}_{j}")
                    nc.sync.dma_start(wl_t[:],
                                      muwl_d[j, :, 512 * oc:512 * oc + 512])
                    for p in range(2):
                        lh = hfT_h[p][:, 128 * j:128 * j + 128]
                        ll = hfT_l[p][:, 128 * j:128 * j + 128]
                        nc.tensor.matmul(mains[p][:], lh, wh_t[:],
                                         start=(j == 0), stop=(j == 31))
                        nc.tensor.matmul(crosses[p][:], lh, wl_t[:],
                                         start=(j == 0), stop=False)
                        nc.tensor.matmul(crosses[p][:], ll, wh_t[:],
                                         start=False, stop=(j == 31))
                for p in range(2):
                    tmp = qp.tile([128, 512], F32, tag="crosstmp", bufs=2,
                                  name=f"ctm_{oc}{p}")
                    nc.vector.tensor_scalar(tmp[:], crosses[p][:],
                                            1.0 / 2048.0, None, ALU.mult)
                    for b in range(NB):
                        msub = qp.tile([65, 512], F32, tag="msub", bufs=8,
                                       name=f"msub{b}{p}_{oc}")
                        nc.vector.memset(msub[64:65, :], 1.0)
                        nc.vector.tensor_tensor(
                            msub[0:64, :],
                            mains[p][64 * b:64 * b + 64, :],
                            tmp[64 * b:64 * b + 64, :], ALU.add)
                        # |lat|^2 accumulation (row sums of squares)
                        sqt = qp.tile([64, 512], F32, tag="sqt", bufs=2,
                                      name=f"sqt{b}{p}_{oc}")
                        nc.scalar.activation(
                            sqt[:], msub[0:64, :], AF.Square,
                            accum_out=sseA[(b, p)][:, oc:oc + 1])
                        # scores + argmax for the 4 pos-chunks of this oc
                        for sc in range(4):
                            pss = ps.tile([128, 512], F32, tag="psC", bufs=2,
                                          name=f"pss{b}{p}{oc}{sc}")
                            nc.tensor.matmul(
                                pss[:], msub[0:65, 128 * sc:128 * sc + 128],
                                embx[0:65, :], start=True, stop=True)
                            vmax = qp.tile([128, 8], F32, tag="vmax", bufs=2,
                                           name=f"vmax{b}{p}{oc}{sc}")
                            vidx = qp.tile([128, 8], U32, tag="vidx", bufs=2,
                                           name=f"vidx{b}{p}{oc}{sc}")
                            nc.vector.max_with_indices(vmax[:], vidx[:],
                                                       pss[:])
                            cidx = 4 * oc + sc
                            nc.vector.tensor_copy(
                                idxcol[(b, p)][:, cidx:cidx + 1],
                                vidx[:, 0:1])
                            nc.vector.tensor_copy(
                                smaxcol[(b, p)][:, cidx:cidx + 1],
                                vmax[:, 0:1])

            # =================== c matmul (fp16 x1) ===================
            for oc in range(8):
                pcs = [ps.tile([128, 512], F32, tag="psA", bufs=2,
                               name=f"cpsm{p}_{oc}") for p in range(2)]
                for j in range(32):
                    cw_t = sp.tile([128, 512], F16, tag="cw", bufs=4,
                                   name=f"cw_{oc}_{j}")
                    nc.sync.dma_start(cw_t[:],
                                      cw_d[j, :, 512 * oc:512 * oc + 512])
                    for p in range(2):
                        lh = hfT_h[p][:, 128 * j:128 * j + 128]
                        nc.tensor.matmul(pcs[p][:], lh, cw_t[:],
                                         start=(j == 0), stop=(j == 31))
                for p in range(2):
                    for b in range(NB):
                        ctmp = qp.tile([64, 512], F32, tag="ctmp", bufs=2,
                                       name=f"ctp{b}{p}_{oc}")
                        nc.scalar.activation(ctmp[:],
                                             pcs[p][64 * b:64 * b + 64, :],
                                             AF.Copy)
                        dst = c_d.ap().rearrange(
                            "b (h two) o -> b h two o",
                            two=2)[b, :, p, 512 * oc:512 * oc + 512]
                        nc.gpsimd.dma_start(dst, ctmp[:])

            # =================== gather (quantized output) ===============
            for b in range(NB):
                for p in range(2):
                    bp = 2 * b + p
                    psT2 = ps.tile([32, 128], F32, tag="psD", bufs=2,
                                   name=f"psT2_{bp}")
                    nc.tensor.transpose(psT2[:], idxcol[(b, p)][:], ident[:])
                    idxT16 = qp.tile([32, 128], F16, tag="idxT16", bufs=2,
                                     name=f"idxT16_{bp}")
                    nc.vector.tensor_copy(idxT16[:], psT2[:])
                    nc.gpsimd.dma_start(idx_scr[bp, :], idxT16[:])
                    idxrow16 = qp.tile([1, 4096], F16, tag="idxrow16",
                                       name=f"idxrow16_{bp}")
                    nc.gpsimd.dma_start(idxrow16[:], idx_scr[bp:bp + 1, :])
                    nc.gpsimd.dma_start(smax_d[bp, :, :], smaxcol[(b, p)][:])
                    nc.gpsimd.dma_start(sse_d[bp, :, :], sseA[(b, p)][:])
                    for c2 in range(8):
                        psb = ps.tile([128, 512], F32, tag="psC", bufs=2,
                                      name=f"psbq{bp}{c2}")
                        nc.tensor.matmul(
                            psb[:], ones16[:],
                            idxrow16[0:1, 512 * c2:512 * c2 + 512],
                            start=True, stop=True)
                        qps = ps.tile([64, 512], F32, tag="psD", bufs=2,
                                      name=f"qps{bp}{c2}")
                        for kt in range(4):
                            oh = qp.tile([128, 512], F16, tag="oh", bufs=2,
                                         name=f"oh{bp}{c2}{kt}")
                            nc.vector.tensor_scalar(oh[:], psb[:],
                                                    kio[:, kt:kt + 1], None,
                                                    ALU.is_equal)
                            nc.tensor.matmul(qps[:], embg[:, kt, :], oh[:],
                                             start=(kt == 0), stop=(kt == 3))
                        qtmp = qp.tile([64, 512], F32, tag="qtmp", bufs=2,
                                       name=f"qtmp{bp}{c2}")
                        nc.scalar.activation(qtmp[:], qps[:], AF.Copy)
                        dst = q_d.ap().rearrange(
                            "b (h two) o -> b h two o",
                            two=2)[b, :, p, 512 * c2:512 * c2 + 512]
                        nc.gpsimd.dma_start(dst, qtmp[:])

    nc.compile()
    return nc


def _prep_inputs(x, params):
    p = {k: np.asarray(v, np.float32) for k, v in params.items()}
    assert not np.any(p["mu_b"]) and not np.any(p["c_b"]), \
        "nonzero mu_b/c_b not supported"

    muT = np.ascontiguousarray(p["mu_w"].T)            # [4096 i, 4096 o]
    muwh = muT.astype(np.float16)
    muwl = ((muT - muwh.astype(np.float32)) * 2048.0).astype(np.float16)
    cT = np.ascontiguousarray(p["c_w"].T).astype(np.float16)
    emb = p["emb"]                                     # [512, 64]
    embx = np.concatenate([emb.T, -0.5 * np.sum(emb * emb, 1)[None, :]],
                          axis=0).astype(np.float32)   # [65, 512]
    embg = np.ascontiguousarray(
        emb.reshape(4, 128, 64).transpose(1, 0, 2)).astype(np.float16)
    kio = np.ascontiguousarray(
        np.arange(128, dtype=np.float32)[:, None]
        + 128.0 * np.arange(4, dtype=np.float32)[None, :])

    shared = {
        "w_in": np.ascontiguousarray(p["in_w"][:, 0, 0, :].T),       # [4, 32]
        "w_hd": np.ascontiguousarray(p["hd_w"][:, :, 0, :].transpose(2, 1, 0)),
        "w_hd2": np.ascontiguousarray(p["hd2_w"][:, :, 0, :].transpose(2, 1, 0)),
        "w_r01": np.ascontiguousarray(p["r0_w1"][:, :, 0, :].transpose(2, 1, 0)),
        "w_r02": np.ascontiguousarray(
            p["r0_w2"][:, :, 0, 0].T.reshape(2, 128, 64)),
        "w_r11": np.ascontiguousarray(p["r1_w1"][:, :, 0, :].transpose(2, 1, 0)),
        "w_r12": np.ascontiguousarray(
            p["r1_w2"][:, :, 0, 0].T.reshape(2, 128, 64)),
        "b_in": np.ascontiguousarray(p["in_b"][:, None]),
        "b_hd": np.ascontiguousarray(p["hd_b"][:, None]),
        "b_hd2": np.ascontiguousarray(p["hd2_b"][:, None]),
        "muwh": np.ascontiguousarray(muwh.reshape(32, 128, 4096)),
        "muwl": np.ascontiguousarray(muwl.reshape(32, 128, 4096)),
        "cw": np.ascontiguousarray(cT.reshape(32, 128, 4096)),
        "embx": embx,
        "embg": embg,
        "kio": kio,
        "ident": np.eye(128, dtype=np.float32),
    }
    x = np.asarray(x, np.float32)
    in_maps = []
    for k in range(NCORES):
        m = dict(shared)
        m["x"] = np.ascontiguousarray(x[2 * k:2 * k + 2])
        in_maps.append(m)
    return in_maps


def kernel(x, params):
    global _NC
    if _NC is None:
        _NC = _build_nc()
    in_maps = _prep_inputs(x, params)
    res = run_bass_kernel_spmd(_NC, in_maps, list(range(NCORES)),
                               trace=TRACE[0])
    kernel.last_exec_time_ns = res.exec_time_ns
    kernel.last_res = res
    q = np.concatenate([res.results[k]["q_out"] for k in range(NCORES)],
                       axis=0)
    c = np.concatenate([res.results[k]["c_out"] for k in range(NCORES)],
                       axis=0)
    sse = 0.0
    for k in range(NCORES):
        sse += float(res.results[k]["sse_out"].sum())
        sse -= 2.0 * float(res.results[k]["smax_out"].sum())
    loss = np.float32(2.0 * sse / (B * H * 64 * 64))
    return q, loss, c


# revision 14
# speedup vs baseline: 1.0782x; 1.0707x over previous
"""Trainium2 Bass kernel for nn_Encoder_conv (conv stack -> big matmuls -> VQ).

Sharding: data-parallel over batch B=16 across 8 cores (2 images/core),
weights replicated, zero cross-core communication, single SPMD launch.

Precision plan (validated against fp64 CPU sim: 0 argmin flips):
  - conv stack: fp32 matmuls (PE 4-pass)
  - mu matmul: fp16 3-pass split (hi + 2^-11-scaled lo), error ~2^-22
  - c matmul:  fp16 single pass (c output tolerance is loose)
  - VQ scores: fp32 matmul, -|e|^2/2 bias folded via ones-row (K=65)
  - gather:    one-hot fp16 matmul (exact 0/1 weights, fp16-rounded emb)
  - loss:      sum over pos of (|lat|^2 - 2*s_max), assembled on host
"""
import sys
import types

import numpy as np

# --- optional NTFF profiling hook shim (antenv.axon_hooks missing in image) ---
try:
    import antenv.axon_hooks  # noqa: F401
except ImportError:
    try:
        import antenv
        _hooks_mod = types.ModuleType("antenv.axon_hooks")
        _hook = [None]
        _hooks_mod.set_axon_ntff_profile_hook = lambda h: _hook.__setitem__(0, h)
        _hooks_mod.get_axon_ntff_profile_hook = lambda: _hook[0]
        sys.modules["antenv.axon_hooks"] = _hooks_mod
        antenv.axon_hooks = _hooks_mod
        from trn_agent_boot.trn_boot import _ntff_profile_via_ctypes
        _hooks_mod.set_axon_ntff_profile_hook(
            _ntff_profile_via_ctypes('/opt/axon/libaxon_pjrt.so'))
    except Exception:
        pass

import concourse.mybir as mybir
import concourse.tile as tile
from concourse import bacc
from concourse.bass_utils import run_bass_kernel_spmd

F32 = mybir.dt.float32
F16 = mybir.dt.float16
U32 = mybir.dt.uint32
AF = mybir.ActivationFunctionType
ALU = mybir.AluOpType

B, H, W = 16, 128, 256
NB = 2          # images per core
NCORES = 8
GH = 32         # hh rows per conv group (4 groups per image)

_NC = None      # cached compiled kernel
TRACE = [False]


def _build_nc():
    nc = bacc.Bacc(target_bir_lowering=False)

    x_d = nc.dram_tensor("x", [2, NB, H, W], F16, kind="ExternalInput")
    w_in_d = nc.dram_tensor("w_in", [2, 4, 32], F16, kind="ExternalInput")
    w_hd_d = nc.dram_tensor("w_hd", [2, 4, 32, 64], F16, kind="ExternalInput")
    w_hd2_d = nc.dram_tensor("w_hd2", [2, 3, 64, 64], F16, kind="ExternalInput")
    w_r01_d = nc.dram_tensor("w_r01", [2, 3, 64, 256], F16, kind="ExternalInput")
    w_r02_d = nc.dram_tensor("w_r02", [2, 2, 128, 64], F16, kind="ExternalInput")
    w_r11_d = nc.dram_tensor("w_r11", [2, 3, 64, 256], F16, kind="ExternalInput")
    w_r12_d = nc.dram_tensor("w_r12", [2, 2, 128, 64], F16, kind="ExternalInput")
    b_in_d = nc.dram_tensor("b_in", [32, 1], F32, kind="ExternalInput")
    b_hd_d = nc.dram_tensor("b_hd", [64, 1], F32, kind="ExternalInput")
    b_hd2_d = nc.dram_tensor("b_hd2", [64, 1], F32, kind="ExternalInput")
    muwh_d = nc.dram_tensor("muwh", [32, 128, 4096], F16, kind="ExternalInput")
    muwl_d = nc.dram_tensor("muwl", [32, 128, 4096], F16, kind="ExternalInput")
    cw_d = nc.dram_tensor("cw", [32, 128, 4096], F16, kind="ExternalInput")
    embx_d = nc.dram_tensor("embx", [65, 512], F32, kind="ExternalInput")
    embg_d = nc.dram_tensor("embg", [128, 4, 64], F16, kind="ExternalInput")
    kio_d = nc.dram_tensor("kio", [128, 4], F32, kind="ExternalInput")
    id_d = nc.dram_tensor("ident", [128, 128], F32, kind="ExternalInput")

    q_d = nc.dram_tensor("q_out", [NB, H, 4096], F32, kind="ExternalOutput")
    c_d = nc.dram_tensor("c_out", [NB, H, 4096], F32, kind="ExternalOutput")
    sse_d = nc.dram_tensor("sse_out", [4, 64, 8], F32, kind="ExternalOutput")
    smax_d = nc.dram_tensor("smax_out", [4, 128, 32], F32,
                            kind="ExternalOutput")
    idx_scr = nc.dram_tensor("idx_scr", [4, 4096], F16)

    with tile.TileContext(nc) as tc:
        with (
            tc.tile_pool(name="const", bufs=1) as cp,
            tc.tile_pool(name="conv", bufs=1) as vp,
            tc.tile_pool(name="persist", bufs=1) as pp,
            tc.tile_pool(name="stream", bufs=1) as sp,
            tc.tile_pool(name="work", bufs=1) as qp,
            tc.tile_pool(name="ps", bufs=1, space="PSUM") as ps,
        ):
            # ---- constants ----
            w_in = cp.tile([4, 2, 32], F16, tag="w_in")
            nc.sync.dma_start(w_in[:], w_in_d.ap().rearrange("s d m -> d s m"))
            w_hd = cp.tile([32, 2, 4, 64], F16, tag="w_hd")
            nc.sync.dma_start(w_hd[:], w_hd_d.ap().rearrange("s d k m -> k s d m"))
            w_hd2 = cp.tile([64, 2, 3, 64], F16, tag="w_hd2")
            nc.sync.dma_start(w_hd2[:], w_hd2_d.ap().rearrange("s d k m -> k s d m"))
            w_r01 = cp.tile([64, 2, 3, 256], F16, tag="w_r01")
            nc.sync.dma_start(w_r01[:], w_r01_d.ap().rearrange("s d k m -> k s d m"))
            w_r02 = cp.tile([128, 2, 2, 64], F16, tag="w_r02")
            nc.sync.dma_start(w_r02[:], w_r02_d.ap().rearrange("s t k m -> k s t m"))
            w_r11 = cp.tile([64, 2, 3, 256], F16, tag="w_r11")
            nc.sync.dma_start(w_r11[:], w_r11_d.ap().rearrange("s d k m -> k s d m"))
            w_r12 = cp.tile([128, 2, 2, 64], F16, tag="w_r12")
            nc.sync.dma_start(w_r12[:], w_r12_d.ap().rearrange("s t k m -> k s t m"))
            b_in = cp.tile([32, 1], F32, tag="b_in")
            nc.sync.dma_start(b_in[:], b_in_d[:])
            b_hd = cp.tile([64, 1], F32, tag="b_hd")
            nc.sync.dma_start(b_hd[:], b_hd_d[:])
            b_hd2 = cp.tile([64, 1], F32, tag="b_hd2")
            nc.sync.dma_start(b_hd2[:], b_hd2_d[:])
            embx = cp.tile([65, 512], F32, tag="embx")
            nc.sync.dma_start(embx[:], embx_d[:])
            embg = cp.tile([128, 4, 64], F16, tag="embg")
            nc.sync.dma_start(embg[:], embg_d[:])
            kio = cp.tile([128, 4], F32, tag="kio")
            nc.sync.dma_start(kio[:], kio_d[:])
            ident = cp.tile([128, 128], F32, tag="ident")
            nc.sync.dma_start(ident[:], id_d[:])
            ones16 = cp.tile([1, 128], F16, tag="ones16")
            nc.vector.memset(ones16[:], 1.0)

            # ---- persistent accumulators ----
            hfT_h = [pp.tile([128, 4096], F16, tag=f"hfTh{p}",
                             name=f"hfTh{p}") for p in range(2)]
            hfT_l = [pp.tile([128, 4096], F16, tag=f"hfTl{p}",
                             name=f"hfTl{p}") for p in range(2)]
            idxcol, smaxcol, sseA = {}, {}, {}
            for b in range(NB):
                for p in range(2):
                    idxcol[(b, p)] = pp.tile([128, 32], F32,
                                             tag=f"idxcol{b}{p}",
                                             name=f"idxcol{b}{p}")
                    smaxcol[(b, p)] = pp.tile([128, 32], F32,
                                              tag=f"smaxcol{b}{p}",
                                              name=f"smaxcol{b}{p}")
                    sseA[(b, p)] = pp.tile([64, 8], F32, tag=f"sseA{b}{p}",
                                           name=f"sseA{b}{p}")

            # =================== conv stack (fp16 x3, unscaled lo) ==========
            for b in range(NB):
                for g in range(4):
                    hh0 = GH * g
                    # --- in-conv (ci=1, k=4, s=2): im2col on partitions ---
                    inxh = vp.tile([4, GH * 260], F16, tag="inxh")
                    inxl = vp.tile([4, GH * 260], F16, tag="inxl")
                    for t_, buf in ((0, inxh), (1, inxl)):
                        br = buf[:].rearrange("p (h w) -> p h w", h=GH, w=260)
                        nc.vector.memset(br[:, :, 0:4], 0.0)
                        nc.vector.memset(br[:, :, 256:260], 0.0)
                        for dw in range(4):
                            nc.sync.dma_start(
                                br[dw:dw + 1, :, (4 - dw):(4 - dw) + 256],
                                x_d[t_, b:b + 1, hh0:hh0 + GH, :])
                    inxh_r = inxh[:].rearrange("p (h w) -> p h w", h=GH, w=260)
                    inxl_r = inxl[:].rearrange("p (h w) -> p h w", h=GH, w=260)
                    h1h = vp.tile([32, GH * 130], F16, tag="h1h")
                    h1l = vp.tile([32, GH * 130], F16, tag="h1l")
                    h1h_r = h1h[:].rearrange("p (h w) -> p h w", h=GH, w=130)
                    h1l_r = h1l[:].rearrange("p (h w) -> p h w", h=GH, w=130)
                    for br in (h1h_r, h1l_r):
                        nc.vector.memset(br[:, :, 0:1], 0.0)
                        nc.vector.memset(br[:, :, 129:130], 0.0)
                    for ch in range(8):   # 4 hh x 128 w' = 512
                        psa = ps.tile([32, 512], F32, tag="psA", bufs=2,
                                      name=f"cv1_{b}{g}{ch}")
                        rhsh = inxh_r[:, 4 * ch:4 * ch + 4, 3:258:2]
                        rhsl = inxl_r[:, 4 * ch:4 * ch + 4, 3:258:2]
                        nc.tensor.matmul(psa[:], w_in[:, 0, :], rhsh,
                                         start=True, stop=False)
                        nc.tensor.matmul(psa[:], w_in[:, 1, :], rhsh,
                                         start=False, stop=False)
                        nc.tensor.matmul(psa[:], w_in[:, 0, :], rhsl,
                                         start=False, stop=True)
                        r32 = qp.tile([128, 512], F32, tag="r32", bufs=2,
                                      name=f"r32a_{b}{g}{ch}")
                        nc.scalar.activation(r32[0:32, :], psa[:], AF.Relu,
                                             bias=b_in[:, 0:1])
                        oh_ = h1h_r[:, 4 * ch:4 * ch + 4, 1:129]
                        ol_ = h1l_r[:, 4 * ch:4 * ch + 4, 1:129]
                        nc.scalar.activation(oh_, r32[0:32, :], AF.Copy)
                        nc.vector.tensor_tensor(ol_, r32[0:32, :], oh_,
                                                ALU.subtract)
                    # --- hd conv (32->64, k=4, s=2) ---
                    h2h = vp.tile([64, GH * 66], F16, tag="h2h")
                    h2l = vp.tile([64, GH * 66], F16, tag="h2l")
                    h2h_r = h2h[:].rearrange("p (h w) -> p h w", h=GH, w=66)
                    h2l_r = h2l[:].rearrange("p (h w) -> p h w", h=GH, w=66)
                    for br in (h2h_r, h2l_r):
                        nc.vector.memset(br[:, :, 0:1], 0.0)
                        nc.vector.memset(br[:, :, 65:66], 0.0)
                    for ch in range(4):   # 8 hh x 64 w' = 512
                        psa = ps.tile([64, 512], F32, tag="psA", bufs=2,
                                      name=f"cv2_{b}{g}{ch}")
                        for dw in range(4):
                            rhsh = h1h_r[:, 8 * ch:8 * ch + 8, dw:dw + 127:2]
                            rhsl = h1l_r[:, 8 * ch:8 * ch + 8, dw:dw + 127:2]
                            nc.tensor.matmul(psa[:], w_hd[:, 0, dw, :], rhsh,
                                             start=(dw == 0), stop=False)
                            nc.tensor.matmul(psa[:], w_hd[:, 1, dw, :], rhsh,
                                             start=False, stop=False)
                            nc.tensor.matmul(psa[:], w_hd[:, 0, dw, :], rhsl,
                                             start=False, stop=(dw == 3))
                        r32 = qp.tile([128, 512], F32, tag="r32", bufs=2,
                                      name=f"r32b_{b}{g}{ch}")
                        nc.scalar.activation(r32[0:64, :], psa[:], AF.Relu,
                                             bias=b_hd[:, 0:1])
                        oh_ = h2h_r[:, 8 * ch:8 * ch + 8, 1:65]
                        ol_ = h2l_r[:, 8 * ch:8 * ch + 8, 1:65]
                        nc.scalar.activation(oh_, r32[0:64, :], AF.Copy)
                        nc.vector.tensor_tensor(ol_, r32[0:64, :], oh_,
                                                ALU.subtract)
                    # --- hd2 conv (64->64, k=3, s=1) ---
                    h3f = vp.tile([64, GH * 66], F32, tag="hB",
                                  name=f"h3f_{b}{g}")
                    h3f_r = h3f[:].rearrange("p (h w) -> p h w", h=GH, w=66)
                    h3h = vp.tile([64, GH * 66], F16, tag="h3h")
                    h3l = vp.tile([64, GH * 66], F16, tag="h3l")
                    h3h_r = h3h[:].rearrange("p (h w) -> p h w", h=GH, w=66)
                    h3l_r = h3l[:].rearrange("p (h w) -> p h w", h=GH, w=66)
                    for br in (h3h_r, h3l_r):
                        nc.vector.memset(br[:, :, 0:1], 0.0)
                        nc.vector.memset(br[:, :, 65:66], 0.0)
                    for ch in range(4):
                        psa = ps.tile([64, 512], F32, tag="psA", bufs=2,
                                      name=f"cv3_{b}{g}{ch}")
                        for dw in range(3):
                            rhsh = h2h_r[:, 8 * ch:8 * ch + 8, dw:dw + 64]
                            rhsl = h2l_r[:, 8 * ch:8 * ch + 8, dw:dw + 64]
                            nc.tensor.matmul(psa[:], w_hd2[:, 0, dw, :], rhsh,
                                             start=(dw == 0), stop=False)
                            nc.tensor.matmul(psa[:], w_hd2[:, 1, dw, :], rhsh,
                                             start=False, stop=False)
                            nc.tensor.matmul(psa[:], w_hd2[:, 0, dw, :], rhsl,
                                             start=False, stop=(dw == 2))
                        o32 = h3f_r[:, 8 * ch:8 * ch + 8, 1:65]
                        nc.scalar.activation(o32, psa[:], AF.Relu,
                                             bias=b_hd2[:, 0:1])
                        oh_ = h3h_r[:, 8 * ch:8 * ch + 8, 1:65]
                        ol_ = h3l_r[:, 8 * ch:8 * ch + 8, 1:65]
                        nc.scalar.activation(oh_, o32, AF.Copy)
                        nc.vector.tensor_tensor(ol_, o32, oh_, ALU.subtract)
                    # --- residual blocks ---
                    cin_h, cin_l = h3h_r, h3l_r   # conv inputs (fp16 pair)
                    res_in_r = h3f_r              # pre-relu running h (fp32)
                    for blk, (w1t, w2t) in enumerate(((w_r01, w_r02),
                                                      (w_r11, w_r12))):
                        last = (blk == 1)
                        y1ah = vp.tile([128, GH * 64], F16, tag="y1ah",
                                       name=f"y1ah_{b}{g}{blk}")
                        y1al = vp.tile([128, GH * 64], F16, tag="y1al",
                                       name=f"y1al_{b}{g}{blk}")
                        y1bh = vp.tile([128, GH * 64], F16, tag="y1bh",
                                       name=f"y1bh_{b}{g}{blk}")
                        y1bl = vp.tile([128, GH * 64], F16, tag="y1bl",
                                       name=f"y1bl_{b}{g}{blk}")
                        for ch in range(4):
                            psa = ps.tile([128, 512], F32, tag="psB", bufs=2,
                                          name=f"rw1a_{b}{g}{blk}{ch}")
                            psb_ = ps.tile([128, 512], F32, tag="psC", bufs=2,
                                           name=f"rw1b_{b}{g}{blk}{ch}")
                            for dw in range(3):
                                rhsh = cin_h[:, 8 * ch:8 * ch + 8, dw:dw + 64]
                                rhsl = cin_l[:, 8 * ch:8 * ch + 8, dw:dw + 64]
                                for pst, mo in ((psa, 0), (psb_, 128)):
                                    nc.tensor.matmul(
                                        pst[:], w1t[:, 0, dw, mo:mo + 128],
                                        rhsh, start=(dw == 0), stop=False)
                                    nc.tensor.matmul(
                                        pst[:], w1t[:, 1, dw, mo:mo + 128],
                                        rhsh, start=False, stop=False)
                                    nc.tensor.matmul(
                                        pst[:], w1t[:, 0, dw, mo:mo + 128],
                                        rhsl, start=False, stop=(dw == 2))
                            for pst, yh, yl in ((psa, y1ah, y1al),
                                                (psb_, y1bh, y1bl)):
                                r32 = qp.tile([128, 512], F32, tag="r32",
                                              bufs=2,
                                              name=f"r32c_{b}{g}{blk}{ch}{yh.name[:4]}")
                                nc.scalar.activation(r32[:], pst[:], AF.Relu)
                                oh_ = yh[:, 512 * ch:512 * ch + 512]
                                ol_ = yl[:, 512 * ch:512 * ch + 512]
                                nc.scalar.activation(oh_, r32[:], AF.Copy)
                                nc.vector.tensor_tensor(ol_, r32[:], oh_,
                                                        ALU.subtract)
                        if not last:
                            hout = vp.tile([64, GH * 66], F32, tag="hA",
                                           name=f"h4_{b}{g}")
                            hout_r = hout[:].rearrange("p (h w) -> p h w",
                                                       h=GH, w=66)
                            nc.vector.memset(hout_r[:, :, 0:1], 0.0)
                            nc.vector.memset(hout_r[:, :, 65:66], 0.0)
                            h4rh = vp.tile([64, GH * 66], F16, tag="h4rh",
                                           name=f"h4rh_{b}{g}")
                            h4rl = vp.tile([64, GH * 66], F16, tag="h4rl",
                                           name=f"h4rl_{b}{g}")
                            h4rh_r = h4rh[:].rearrange("p (h w) -> p h w",
                                                       h=GH, w=66)
                            h4rl_r = h4rl[:].rearrange("p (h w) -> p h w",
                                                       h=GH, w=66)
                            for br in (h4rh_r, h4rl_r):
                                nc.vector.memset(br[:, :, 0:1], 0.0)
                                nc.vector.memset(br[:, :, 65:66], 0.0)
                        else:
                            h5r = vp.tile([64, GH * 64], F32, tag="h5r",
                                          name=f"h5r_{b}{g}")
                        for ch in range(4):
                            psa = ps.tile([64, 512], F32, tag="psA", bufs=2,
                                          name=f"rw2_{b}{g}{blk}{ch}")
                            for t_, yh, yl in ((0, y1ah, y1al),
                                               (1, y1bh, y1bl)):
                                yhs = yh[:, 512 * ch:512 * ch + 512]
                                yls = yl[:, 512 * ch:512 * ch + 512]
                                nc.tensor.matmul(psa[:], w2t[:, 0, t_, :],
                                                 yhs, start=(t_ == 0),
                                                 stop=False)
                                nc.tensor.matmul(psa[:], w2t[:, 1, t_, :],
                                                 yhs, start=False, stop=False)
                                nc.tensor.matmul(psa[:], w2t[:, 0, t_, :],
                                                 yls, start=False,
                                                 stop=(t_ == 1))
                            psv = psa[:].rearrange("p (a w) -> p a w",
                                                   a=8, w=64)
                            hin_i = res_in_r[:, 8 * ch:8 * ch + 8, 1:65]
                            if not last:
                                out = hout_r[:, 8 * ch:8 * ch + 8, 1:65]
                                nc.vector.tensor_tensor(out, psv, hin_i,
                                                        ALU.add)
                                r32 = qp.tile([128, 512], F32, tag="r32",
                                              bufs=2, name=f"r32d_{b}{g}{ch}")
                                r32v = r32[0:64, :].rearrange(
                                    "p (a w) -> p a w", a=8, w=64)
                                nc.scalar.activation(r32v, out, AF.Relu)
                                oh_ = h4rh_r[:, 8 * ch:8 * ch + 8, 1:65]
                                ol_ = h4rl_r[:, 8 * ch:8 * ch + 8, 1:65]
                                nc.scalar.activation(oh_, r32v, AF.Copy)
                                nc.vector.tensor_tensor(ol_, r32v, oh_,
                                                        ALU.subtract)
                            else:
                                out = h5r[:, 512 * ch:512 * ch + 512]
                                outv = out.rearrange("p (a w) -> p a w",
                                                     a=8, w=64)
                                nc.vector.tensor_tensor(outv, psv, hin_i,
                                                        ALU.add)
                                nc.scalar.activation(out, out, AF.Relu)
                        if not last:
                            cin_h, cin_l = h4rh_r, h4rl_r
                            res_in_r = hout_r
                    # --- transposes for this group -> hfT (p = g//2) ---
                    p = g // 2
                    j0 = 16 * (g % 2)
                    for t in range(16):
                        psT = ps.tile([128, 64], F32, tag="psD", bufs=2,
                                      name=f"psT_{b}{g}{t}")
                        nc.tensor.transpose(psT[:],
                                            h5r[:, 128 * t:128 * t + 128],
                                            ident[0:64, 0:64])
                        j = j0 + t
                        dsth = hfT_h[p][:, 128 * j + 64 * b:
                                        128 * j + 64 * b + 64]
                        dstl = hfT_l[p][:, 128 * j + 64 * b:
                                        128 * j + 64 * b + 64]
                        nc.scalar.activation(dsth, psT[:], AF.Copy)
                        tmp = qp.tile([128, 64], F32, tag="splittmp", bufs=1,
                                      name=f"spl_{b}{g}{t}")
                        nc.vector.tensor_tensor(tmp[:], psT[:], dsth,
                                                ALU.subtract)
                        nc.vector.tensor_scalar(dstl, tmp[:], 2048.0, None,
                                                ALU.mult)

            # ============ mu matmul (fp16 x3) + VQ scores per o-chunk ======
            for oc in range(8):
                mains = [ps.tile([128, 512], F32, tag="psA", bufs=2,
                                 name=f"mumain{p}_{oc}") for p in range(2)]
                crosses = [ps.tile([128, 512], F32, tag="psB", bufs=2,
                                   name=f"mucross{p}_{oc}") for p in range(2)]
                for j in range(32):
                    wh_t = sp.tile([128, 512], F16, tag="wh", bufs=4,
                                   name=f"wh_{oc}_{j}")
                    nc.sync.dma_start(wh_t[:],
                                      muwh_d[j, :, 512 * oc:512 * oc + 512])
                    wl_t = sp.tile([128, 512], F16, tag="wl", bufs=4,
                                   name=f"wl_{oc}_{j}")
                    nc.scalar.dma_start(wl_t[:],
                                        muwl_d[j, :, 512 * oc:512 * oc + 512])
                    for p in range(2):
                        lh = hfT_h[p][:, 128 * j:128 * j + 128]
                        ll = hfT_l[p][:, 128 * j:128 * j + 128]
                        nc.tensor.matmul(mains[p][:], lh, wh_t[:],
                                         start=(j == 0), stop=(j == 31))
                        nc.tensor.matmul(crosses[p][:], lh, wl_t[:],
                                         start=(j == 0), stop=False)
                        nc.tensor.matmul(crosses[p][:], ll, wh_t[:],
                                         start=False, stop=(j == 31))
                for p in range(2):
                    tmp = qp.tile([128, 512], F32, tag="crosstmp", bufs=1,
                                  name=f"ctm_{oc}{p}")
                    nc.vector.tensor_scalar(tmp[:], crosses[p][:],
                                            1.0 / 2048.0, None, ALU.mult)
                    for b in range(NB):
                        msub = qp.tile([65, 512], F32, tag="msub", bufs=5,
                                       name=f"msub{b}{p}_{oc}")
                        nc.vector.memset(msub[64:65, :], 1.0)
                        nc.vector.tensor_tensor(
                            msub[0:64, :],
                            mains[p][64 * b:64 * b + 64, :],
                            tmp[64 * b:64 * b + 64, :], ALU.add)
                        # |lat|^2 accumulation (row sums of squares)
                        sqt = qp.tile([64, 512], F32, tag="sqt", bufs=1,
                                      name=f"sqt{b}{p}_{oc}")
                        nc.scalar.activation(
                            sqt[:], msub[0:64, :], AF.Square,
                            accum_out=sseA[(b, p)][:, oc:oc + 1])
                        # scores + argmax for the 4 pos-chunks of this oc
                        for sc in range(4):
                            pss = ps.tile([128, 512], F32, tag="psC", bufs=2,
                                          name=f"pss{b}{p}{oc}{sc}")
                            nc.tensor.matmul(
                                pss[:], msub[0:65, 128 * sc:128 * sc + 128],
                                embx[0:65, :], start=True, stop=True)
                            vmax = qp.tile([128, 8], F32, tag="vmax", bufs=2,
                                           name=f"vmax{b}{p}{oc}{sc}")
                            vidx = qp.tile([128, 8], U32, tag="vidx", bufs=2,
                                           name=f"vidx{b}{p}{oc}{sc}")
                            nc.vector.max_with_indices(vmax[:], vidx[:],
                                                       pss[:])
                            cidx = 4 * oc + sc
                            nc.vector.tensor_copy(
                                idxcol[(b, p)][:, cidx:cidx + 1],
                                vidx[:, 0:1])
                            nc.vector.tensor_copy(
                                smaxcol[(b, p)][:, cidx:cidx + 1],
                                vmax[:, 0:1])

            # =================== c matmul (fp16 x1) ===================
            for oc in range(8):
                pcs = [ps.tile([128, 512], F32, tag="psA", bufs=2,
                               name=f"cpsm{p}_{oc}") for p in range(2)]
                for j in range(32):
                    cw_t = sp.tile([128, 512], F16, tag="cw", bufs=4,
                                   name=f"cw_{oc}_{j}")
                    nc.sync.dma_start(cw_t[:],
                                      cw_d[j, :, 512 * oc:512 * oc + 512])
                    for p in range(2):
                        lh = hfT_h[p][:, 128 * j:128 * j + 128]
                        nc.tensor.matmul(pcs[p][:], lh, cw_t[:],
                                         start=(j == 0), stop=(j == 31))
                for p in range(2):
                    for b in range(NB):
                        ctmp = qp.tile([64, 512], F32, tag="ctmp", bufs=1,
                                       name=f"ctp{b}{p}_{oc}")
                        nc.scalar.activation(ctmp[:],
                                             pcs[p][64 * b:64 * b + 64, :],
                                             AF.Copy)
                        dst = c_d.ap().rearrange(
                            "b (h two) o -> b h two o",
                            two=2)[b, :, p, 512 * oc:512 * oc + 512]
                        nc.gpsimd.dma_start(dst, ctmp[:])

            # =================== gather (quantized output) ===============
            for b in range(NB):
                for p in range(2):
                    bp = 2 * b + p
                    psT2 = ps.tile([32, 128], F32, tag="psD", bufs=2,
                                   name=f"psT2_{bp}")
                    nc.tensor.transpose(psT2[:], idxcol[(b, p)][:], ident[:])
                    idxT16 = qp.tile([32, 128], F16, tag="idxT16", bufs=2,
                                     name=f"idxT16_{bp}")
                    nc.vector.tensor_copy(idxT16[:], psT2[:])
                    nc.gpsimd.dma_start(idx_scr[bp, :], idxT16[:])
                    idxrow16 = qp.tile([1, 4096], F16, tag="idxrow16",
                                       name=f"idxrow16_{bp}")
                    nc.gpsimd.dma_start(idxrow16[:], idx_scr[bp:bp + 1, :])
                    nc.gpsimd.dma_start(smax_d[bp, :, :], smaxcol[(b, p)][:])
                    nc.gpsimd.dma_start(sse_d[bp, :, :], sseA[(b, p)][:])
                    for c2 in range(8):
                        psb = ps.tile([128, 512], F32, tag="psC", bufs=2,
                                      name=f"psbq{bp}{c2}")
                        nc.tensor.matmul(
                            psb[:], ones16[:],
                            idxrow16[0:1, 512 * c2:512 * c2 + 512],
                            start=True, stop=True)
                        qps = ps.tile([64, 512], F32, tag="psD", bufs=2,
                                      name=f"qps{bp}{c2}")
                        for kt in range(4):
                            oh = qp.tile([128, 512], F16, tag="oh", bufs=2,
                                         name=f"oh{bp}{c2}{kt}")
                            nc.vector.tensor_scalar(oh[:], psb[:],
                                                    kio[:, kt:kt + 1], None,
                                                    ALU.is_equal)
                            nc.tensor.matmul(qps[:], embg[:, kt, :], oh[:],
                                             start=(kt == 0), stop=(kt == 3))
                        qtmp = qp.tile([64, 512], F32, tag="qtmp", bufs=2,
                                       name=f"qtmp{bp}{c2}")
                        nc.scalar.activation(qtmp[:], qps[:], AF.Copy)
                        dst = q_d.ap().rearrange(
                            "b (h two) o -> b h two o",
                            two=2)[b, :, p, 512 * c2:512 * c2 + 512]
                        nc.gpsimd.dma_start(dst, qtmp[:])

    nc.compile()
    return nc


def _prep_inputs(x, params):
    p = {k: np.asarray(v, np.float32) for k, v in params.items()}
    assert not np.any(p["mu_b"]) and not np.any(p["c_b"]), \
        "nonzero mu_b/c_b not supported"

    muT = np.ascontiguousarray(p["mu_w"].T)            # [4096 i, 4096 o]
    muwh = muT.astype(np.float16)
    muwl = ((muT - muwh.astype(np.float32)) * 2048.0).astype(np.float16)
    cT = np.ascontiguousarray(p["c_w"].T).astype(np.float16)
    emb = p["emb"]                                     # [512, 64]
    embx = np.concatenate([emb.T, -0.5 * np.sum(emb * emb, 1)[None, :]],
                          axis=0).astype(np.float32)   # [65, 512]
    embg = np.ascontiguousarray(
        emb.reshape(4, 128, 64).transpose(1, 0, 2)).astype(np.float16)
    kio = np.ascontiguousarray(
        np.arange(128, dtype=np.float32)[:, None]
        + 128.0 * np.arange(4, dtype=np.float32)[None, :])

    def pair16(a):
        ah = a.astype(np.float16)
        al = (a - ah.astype(np.float32)).astype(np.float16)
        return np.ascontiguousarray(np.stack([ah, al]))

    shared = {
        "w_in": pair16(p["in_w"][:, 0, 0, :].T),                     # [2,4,32]
        "w_hd": pair16(p["hd_w"][:, :, 0, :].transpose(2, 1, 0)),
        "w_hd2": pair16(p["hd2_w"][:, :, 0, :].transpose(2, 1, 0)),
        "w_r01": pair16(p["r0_w1"][:, :, 0, :].transpose(2, 1, 0)),
        "w_r02": pair16(p["r0_w2"][:, :, 0, 0].T.reshape(2, 128, 64)),
        "w_r11": pair16(p["r1_w1"][:, :, 0, :].transpose(2, 1, 0)),
        "w_r12": pair16(p["r1_w2"][:, :, 0, 0].T.reshape(2, 128, 64)),
        "b_in": np.ascontiguousarray(p["in_b"][:, None]),
        "b_hd": np.ascontiguousarray(p["hd_b"][:, None]),
        "b_hd2": np.ascontiguousarray(p["hd2_b"][:, None]),
        "muwh": np.ascontiguousarray(muwh.reshape(32, 128, 4096)),
        "muwl": np.ascontiguousarray(muwl.reshape(32, 128, 4096)),
        "cw": np.ascontiguousarray(cT.reshape(32, 128, 4096)),
        "embx": embx,
        "embg": embg,
        "kio": kio,
        "ident": np.eye(128, dtype=np.float32),
    }
    x = np.asarray(x, np.float32)
    xh = x.astype(np.float16)
    xl = (x - xh.astype(np.float32)).astype(np.float16)
    in_maps = []
    for k in range(NCORES):
        m = dict(shared)
        m["x"] = np.ascontiguousarray(
            np.stack([xh[2 * k:2 * k + 2], xl[2 * k:2 * k + 2]]))
        in_maps.append(m)
    return in_maps


def kernel(x, params):
    global _NC
    if _NC is None:
        _NC = _build_nc()
    in_maps = _prep_inputs(x, params)
    res = run_bass_kernel_spmd(_NC, in_maps, list(range(NCORES)),
                               trace=TRACE[0])
    kernel.last_exec_time_ns = res.exec_time_ns
    kernel.last_res = res
    q = np.concatenate([res.results[k]["q_out"] for k in range(NCORES)],
                       axis=0)
    c = np.concatenate([res.results[k]["c_out"] for k in range(NCORES)],
                       axis=0)
    sse = 0.0
    for k in range(NCORES):
        sse += float(res.results[k]["sse_out"].sum())
        sse -= 2.0 * float(res.results[k]["smax_out"].sum())
    loss = np.float32(2.0 * sse / (B * H * 64 * 64))
    return q, loss, c
